# revision 1
# baseline (speedup 1.0000x reference)
"""Trainium2 Bass kernel for an 8-head attention layer + FFN (B=2, S=2048,
D=1024, DQK=128, DFF=4096), distributed over 8 NeuronCores.

Sharding: head-parallel attention (1 head per core). For each 512-token
group, a bf16 ReduceScatter (D-sliced across cores) is issued as soon as
the group's attention output is stored, hiding the collective under the
next group's compute; one AllToAll at the end re-assembles each core's own
token group, then the FFN runs token-parallel (512 tokens per core).

The attention loop is interleaved per 512-token chunk: projections for
chunk t are immediately followed by the two 256-wide s-chunks whose causal
window chunk t completes, so DMA waits overlap matmul work and each
ReduceScatter fires as early as possible. All DMAs are issued from the two
HWDGE engines (sync/scalar) — gpsimd DMA is software-DGE and slow. FFN W1
is prefetched SBUF-resident during batch 0; W2 streams fo-major during
FFN2, which holds all 8 output PSUM banks and accumulates fo-outer.

Precision plan (validated numerically, ~4.2e-3 max rel err vs 2e-2 gate):
  - q/k projections + scores: bf16 weights/x, f32r scores
  - v projection: fp8(e4m3) DoubleRow for all tokens; an extra bf16
    projection for tokens < 256 used by the first s-chunk (short causal
    softmax rows don't average out fp8 noise)
  - attnV + softmax sums: fp8 DoubleRow for s-chunks >= 1, bf16 for sc 0,
    with the softmax denominator computed from the same quantized e tiles
  - FFN: bf16 weights/activations, f32 accumulate + f32 residuals
  - collective payload: bf16

On-chip layouts keep the contraction dim on partitions throughout; DoubleRow
tiles carry a k-pair axis (pair outermost in the free dims, per the
s3_lw/s3d3_mm dual-fp8 ISA restrictions, with M=128 so col_grp=0xf):
x8 [128, 2, 512], v8 [128, 2, D], e8 [128, 2, SC]. Softmax runs without
max-subtraction (scores are O(1)); column sums come from full-width
ones-matmuls so 1/sum lands already broadcast across all partitions.
"""
import sys

sys.path.insert(0, "/opt/trn_rl_repo")
import numpy as np
import ml_dtypes

B, S, D, H, DQK, DFF = 2, 2048, 1024, 8, 128, 4096
P = 128
SC = 256                 # attention s-chunk width
NSC = S // SC            # s-chunks per batch
TOK = 512                # tokens per core in the FFN phase
NG = (B * S) // TOK      # 8 token groups == 8 cores
NCORES = 8
NT = S // P              # 16 t-blocks
ND = D // P              # 8 d-blocks
ND2 = ND // 2            # 4 d-block pairs (DoubleRow)
NF = DFF // P            # 32 f-blocks
NO = D // P              # 8 output o-blocks
NVB = SC // P            # t-blocks with a bf16 v copy (short rows)
SCALE = 1.0 / float(np.sqrt(DQK))
FFN_BF16 = True          # kept for test.py compat; FFN is always bf16 now
PIPELINED_RS = True

F8 = ml_dtypes.float8_e4m3
BF = ml_dtypes.bfloat16


def _mask_schedule(mask):
    """Classify each (t-block, s-chunk) tile of the score matrix.

    Returns (sched, mtiles): sched[sc] is a list of (bt, mask_idx) where
    mask_idx is None for fully-unmasked tiles; fully-masked tiles are
    dropped. mtiles[i] is a [P, SC] 0/1 fp32 tile multiplied into exp(s)
    (layout [t, s], matching the on-chip scoresT layout).
    """
    mask = np.asarray(mask, dtype=bool)
    sched = []
    uniq = {}
    mtiles = []
    for sc in range(NSC):
        s0 = sc * SC
        entries = []
        for bt in range(NT):
            sub = mask[s0 : s0 + SC, bt * P : (bt + 1) * P]  # [s, t]
            if sub.all():
                continue
            if not sub.any():
                entries.append((bt, None))
                continue
            tileT = np.where(sub.T, np.float32(0.0), np.float32(1.0)).copy()
            key = tileT.tobytes()
            if key not in uniq:
                uniq[key] = len(mtiles)
                mtiles.append(tileT)
            entries.append((bt, uniq[key]))
        sched.append(entries)
    return sched, mtiles


def _bf16_scs(sched):
    """s-chunks forced onto the bf16 path: sc 0 (short softmax rows) plus
    any chunk whose entries aren't a contiguous even-length prefix pairing
    (fp8 DoubleRow pairs t-blocks (2m, 2m+1))."""
    out = set()
    for sc, entries in enumerate(sched):
        bts = [bt for bt, _ in entries]
        if sc == 0 or len(bts) % 2 or bts != list(range(len(bts))):
            out.add(sc)
    return out


def _build(sched, n_mask, collective=True, reps=1):
    import concourse.mybir as mybir
    import concourse.tile as tile
    from concourse import bacc

    F32 = mybir.dt.float32
    F32R = mybir.dt.float32r
    BF16 = mybir.dt.bfloat16
    FP8 = mybir.dt.float8e4
    AF = mybir.ActivationFunctionType
    OP = mybir.AluOpType
    DR = mybir.MatmulPerfMode.DoubleRow

    bf16_scs = _bf16_scs(sched)

    nc = bacc.Bacc("TRN2", target_bir_lowering=False, debug=False,
                   num_devices=NCORES)

    xTb_in = nc.dram_tensor("xTb", [B, D, S], BF16, kind="ExternalInput")
    x8_in = nc.dram_tensor("x8", [B, ND2, P, 2, S], FP8, kind="ExternalInput")
    wqT_in = nc.dram_tensor("wqT", [D, DQK], BF16, kind="ExternalInput")
    wkT_in = nc.dram_tensor("wkT", [D, DQK], BF16, kind="ExternalInput")
    wv8_in = nc.dram_tensor("wv8", [ND2, P, 2, D], FP8, kind="ExternalInput")
    wvTb_in = nc.dram_tensor("wvTb", [D, D], BF16, kind="ExternalInput")
    w1b_in = nc.dram_tensor("w1b", [ND, P, DFF], BF16, kind="ExternalInput")
    w2b_in = nc.dram_tensor("w2b", [NF, P, D], BF16, kind="ExternalInput")
    b1_in = nc.dram_tensor("b1c", [P, NF], F32, kind="ExternalInput")
    b2_in = nc.dram_tensor("b2c", [P, ND], F32, kind="ExternalInput")
    mtb_in = nc.dram_tensor("mtb", [max(n_mask, 1), P, SC], BF16,
                            kind="ExternalInput")
    mt8_in = nc.dram_tensor("mt8", [max(n_mask, 1), P, SC], FP8,
                            kind="ExternalInput")
    onecb_in = nc.dram_tensor("onecb", [P, P], BF16, kind="ExternalInput")
    one8_in = nc.dram_tensor("one8", [P, 2, P], FP8, kind="ExternalInput")
    xTg_in = nc.dram_tensor("xTg", [D, TOK], F32, kind="ExternalInput")
    outT = nc.dram_tensor("outT", [D, TOK], F32, kind="ExternalOutput")

    xTb_r = xTb_in.rearrange("b (o p) s -> b o p s", p=P)
    wqT_r = wqT_in.rearrange("(o p) e -> o p e", p=P)
    wkT_r = wkT_in.rearrange("(o p) e -> o p e", p=P)
    wvTb_r = wvTb_in.rearrange("(o p) d -> o p d", p=P)
    xTg_r = xTg_in.rearrange("(o p) t -> o p t", p=P)
    outT_r = outT.rearrange("(o p) t -> o p t", p=P)

    with tile.TileContext(nc) as tc:
        with (
            tc.tile_pool(name="consts", bufs=1) as consts,
            tc.tile_pool(name="dram", bufs=1, space="DRAM") as dram,
        ):
            ones_cb = consts.tile([P, P], BF16, tag="onecb")
            nc.sync.dma_start(ones_cb[:], onecb_in[:])
            ones8 = consts.tile([P, 2, P], FP8, tag="one8")
            nc.sync.dma_start(ones8[:], one8_in[:])
            b1_sb = consts.tile([P, NF], F32, tag="b1")
            nc.sync.dma_start(b1_sb[:], b1_in[:])
            b2_sb = consts.tile([P, ND], F32, tag="b2")
            nc.sync.dma_start(b2_sb[:], b2_in[:])
            mtb_sb = []
            mt8_sb = []
            for i in range(n_mask):
                t = consts.tile([P, SC], BF16, tag=f"mtb{i}")
                nc.scalar.dma_start(t[:], mtb_in[i])
                mtb_sb.append(t)
                t8 = consts.tile([P, SC], FP8, tag=f"mt8{i}")
                nc.scalar.dma_start(t8[:], mt8_in[i])
                mt8_sb.append(t8)

            # constant weights, loaded once (SBUF-resident across reps)
            wq_t, wk_t, wv8_t, wvb_t = [], [], [], []
            for do in range(ND):
                wq = consts.tile([P, DQK], BF16, tag=f"wq{do}",
                                 name=f"wq{do}")
                nc.sync.dma_start(wq[:], wqT_r[do])
                wq_t.append(wq)
                wk = consts.tile([P, DQK], BF16, tag=f"wk{do}",
                                 name=f"wk{do}")
                nc.scalar.dma_start(wk[:], wkT_r[do])
                wk_t.append(wk)
            for dp in range(ND2):
                wv8 = consts.tile([P, 2, D], FP8, tag=f"wv8{dp}",
                                  name=f"wv8{dp}")
                wv8_t.append(wv8)
            for do in range(ND):
                wvb = consts.tile([P, D], BF16, tag=f"wvb{do}",
                                  name=f"wvb{do}")
                wvb_t.append(wvb)
            wv_loaded = []
            # w1 resident tiles [P, DFF] per d-block; DMAs emitted
            # mid-attention (first rep) so they don't block the x loads
            w1_t = [consts.tile([P, DFF], BF16, tag=f"w1_{do}",
                                name=f"w1_{do}") for do in range(ND)]
            w1_loaded = []

            def emit_body():
                cc_in = dram.tile([NG, D, TOK], BF16, tag="cc_in",
                                  name="cc_in")
                ccg = dram.tile([NG, P, TOK], BF16, tag="ccg", name="ccg")
                cc_out = dram.tile([D, TOK], BF16, tag="cc_out",
                                   name="cc_out")
                cc_out_r = cc_out.rearrange("(o p) t -> o p t", p=P)

                def emit_group_rs(g):
                    if not collective:
                        return
                    if PIPELINED_RS:
                        nc.gpsimd.collective_compute(
                            "ReduceScatter",
                            mybir.AluOpType.add,
                            replica_groups=[list(range(NCORES))],
                            ins=[cc_in[g].opt()],
                            outs=[ccg[g].opt()],
                        )

                # ---------------- attention (head-parallel) ----------------
                with (
                    tc.tile_pool(name="xt", bufs=3) as xtp,
                    tc.tile_pool(name="qk", bufs=1) as qkp,
                    tc.tile_pool(name="vp", bufs=1) as vp,
                    tc.tile_pool(name="ep", bufs=12) as ep,
                    tc.tile_pool(name="ebp", bufs=6) as ebp,
                    tc.tile_pool(name="rbp", bufs=2) as rbp,
                    tc.tile_pool(name="aop", bufs=4) as aop,
                    tc.tile_pool(name="ps_pr", bufs=2, space="PSUM") as ps_pr,
                    tc.tile_pool(name="ps_sc", bufs=2, space="PSUM") as ps_sc,
                    tc.tile_pool(name="ps_sum", bufs=1, space="PSUM") as ps_sum,
                    tc.tile_pool(name="ps_at", bufs=3, space="PSUM") as ps_at,
                ):
                    for b in range(B):
                        # --- per-512-chunk projections, then the two
                        #     s-chunks whose causal window they complete ---
                        qT_t = [qkp.tile([P, 512], F32R, tag=f"qT{t}",
                                         name=f"qT{t}") for t in range(4)]
                        kT_t = [qkp.tile([P, 512], F32R, tag=f"kT{t}",
                                         name=f"kT{t}") for t in range(4)]
                        v8_t = [vp.tile([P, 2, D], FP8, tag=f"v8_{m}",
                                        name=f"v8_{m}")
                                for m in range(NT // 2)]
                        vb_t = [vp.tile([P, D], BF16, tag=f"vb{ti}",
                                        name=f"vb{ti}") for ti in range(NVB)]
                        for tch in range(4):  # 512-token chunks of S
                            sl = slice(tch * 512, (tch + 1) * 512)
                            xbs = []
                            for do in range(ND):
                                xb = xtp.tile([P, 512], BF16, tag=f"xb{do}")
                                (nc.sync, nc.scalar)[do % 2].dma_start(
                                    xb[:], xTb_r[b, do, :, sl])
                                xbs.append(xb)
                            x8s = []
                            for dp in range(ND2):
                                x8 = xtp.tile([P, 2, 512], FP8, tag=f"x8{dp}")
                                (nc.sync, nc.scalar)[dp % 2].dma_start(
                                    x8[:], x8_in[b, dp, :, :, sl])
                                x8s.append(x8)
                            if not wv_loaded:
                                wv_loaded.append(True)
                                for dp in range(ND2):
                                    (nc.sync, nc.scalar)[dp % 2].dma_start(
                                        wv8_t[dp][:], wv8_in[dp])
                                for do in range(ND):
                                    (nc.sync, nc.scalar)[do % 2].dma_start(
                                        wvb_t[do][:], wvTb_r[do])
                            qps = ps_pr.tile([P, 512], F32, tag="pr")
                            for do in range(ND):
                                nc.tensor.matmul(qps[:], wq_t[do][:], xbs[do][:],
                                                 start=(do == 0),
                                                 stop=(do == ND - 1))
                            nc.vector.tensor_copy(qT_t[tch][:], qps[:])
                            kps = ps_pr.tile([P, 512], F32, tag="pr")
                            for do in range(ND):
                                nc.tensor.matmul(kps[:], wk_t[do][:], xbs[do][:],
                                                 start=(do == 0),
                                                 stop=(do == ND - 1))
                            nc.vector.tensor_copy(kT_t[tch][:], kps[:])
                            for ti in range(4):  # t-blocks within this chunk
                                to = tch * 4 + ti
                                tsl = slice(ti * P, (ti + 1) * P)
                                for oc in range(2):
                                    osl = slice(oc * 512, (oc + 1) * 512)
                                    vps = ps_pr.tile([P, 512], F32, tag="pr")
                                    for dp in range(ND2):
                                        nc.tensor.matmul(
                                            vps[:], x8s[dp][:, :, tsl],
                                            wv8_t[dp][:, :, osl],
                                            start=(dp == 0),
                                            stop=(dp == ND2 - 1),
                                            perf_mode=DR)
                                    nc.vector.tensor_copy(
                                        v8_t[to // 2][:, to % 2, osl], vps[:])
                                    if tch == 0 and ti < NVB:
                                        vbs = ps_pr.tile([P, 512], F32,
                                                         tag="pr")
                                        for do in range(ND):
                                            nc.tensor.matmul(
                                                vbs[:], xbs[do][:, tsl],
                                                wvb_t[do][:, osl],
                                                start=(do == 0),
                                                stop=(do == ND - 1))
                                        nc.scalar.copy(vb_t[ti][:, osl],
                                                       vbs[:])

                            # --- the two s-chunks completed by this tch ---
                            for sc in (2 * tch, 2 * tch + 1):
                                ssl = slice((sc % 2) * SC, (sc % 2 + 1) * SC)
                                entries = sched[sc]
                                use_bf = sc in bf16_scs
                                sums = ps_sum.tile([P, SC], F32, tag="sum")
                                if use_bf:
                                    e_sb = {}
                                    for i, (bt, mi) in enumerate(entries):
                                        sp = ps_sc.tile([P, SC], F32, tag="sc")
                                        nc.tensor.matmul(
                                            sp[:],
                                            kT_t[bt // 4][:, (bt % 4) * P:
                                                          (bt % 4 + 1) * P],
                                            qT_t[tch][:, ssl],
                                            start=True, stop=True)
                                        e = ebp.tile([P, SC], BF16, tag="eb")
                                        nc.scalar.activation(e[:], sp[:],
                                                             AF.Exp,
                                                             scale=SCALE)
                                        if mi is not None:
                                            nc.vector.tensor_tensor(
                                                e[:], e[:], mtb_sb[mi][:],
                                                OP.mult)
                                        e_sb[bt] = e
                                        nc.tensor.matmul(
                                            sums[:], ones_cb[:], e[:],
                                            start=(i == 0),
                                            stop=(i == len(entries) - 1))
                                else:
                                    npair = len(entries) // 2
                                    e_pr = [ep.tile([P, 2, SC], FP8, tag="e8",
                                                    name=f"e8_{m}")
                                            for m in range(npair)]
                                    for i, (bt, mi) in enumerate(entries):
                                        sp = ps_sc.tile([P, SC], F32, tag="sc")
                                        nc.tensor.matmul(
                                            sp[:],
                                            kT_t[bt // 4][:, (bt % 4) * P:
                                                          (bt % 4 + 1) * P],
                                            qT_t[tch][:, ssl],
                                            start=True, stop=True)
                                        ev = e_pr[i // 2][:, i % 2]
                                        nc.scalar.activation(ev, sp[:], AF.Exp,
                                                             scale=SCALE)
                                        if mi is not None:
                                            nc.vector.tensor_tensor(
                                                ev, ev, mt8_sb[mi][:], OP.mult)
                                    for m in range(npair):
                                        nc.tensor.matmul(
                                            sums[:], ones8[:], e_pr[m][:],
                                            start=(m == 0),
                                            stop=(m == npair - 1),
                                            perf_mode=DR)
                                rb_sb = rbp.tile([P, SC], F32R, tag="rb_sb")
                                with nc.allow_low_precision(
                                        reason="softmax 1/sum in f32r"):
                                    nc.vector.reciprocal(rb_sb[:], sums[:])

                                g = (b * S + sc * SC) // TOK
                                off = (sc * SC) % TOK
                                for oc in range(NO):
                                    ap = ps_at.tile([P, SC], F32, tag="at")
                                    ocsl = slice(oc * P, (oc + 1) * P)
                                    if use_bf:
                                        for i, (bt, _mi) in enumerate(entries):
                                            nc.tensor.matmul(
                                                ap[:], vb_t[bt][:, ocsl],
                                                e_sb[bt][:],
                                                start=(i == 0),
                                                stop=(i == len(entries) - 1))
                                    else:
                                        for m in range(npair):
                                            nc.tensor.matmul(
                                                ap[:], v8_t[m][:, :, ocsl],
                                                e_pr[m][:],
                                                start=(m == 0),
                                                stop=(m == npair - 1),
                                                perf_mode=DR)
                                    ao = aop.tile([P, SC], BF16, tag="ao")
                                    nc.vector.tensor_tensor(ao[:], ap[:],
                                                            rb_sb[:], OP.mult)
                                    (nc.sync, nc.scalar)[oc % 2].dma_start(
                                        cc_in[g, ocsl, off:off + SC], ao[:])
                            emit_group_rs(b * 4 + tch)
                            if b == 0 and len(w1_loaded) < ND:
                                for do in (2 * tch, 2 * tch + 1):
                                    w1_loaded.append(True)
                                    (nc.sync, nc.scalar)[do % 2].dma_start(
                                        w1_t[do][:], w1b_in[do])

                # ---------------- collective ----------------
                if collective:
                    if PIPELINED_RS:
                        nc.gpsimd.collective_compute(
                            "AllToAll",
                            mybir.AluOpType.bypass,
                            replica_groups=[list(range(NCORES))],
                            ins=[ccg.opt()],
                            outs=[cc_out.opt()],
                        )
                    else:
                        nc.gpsimd.collective_compute(
                            "ReduceScatter",
                            mybir.AluOpType.add,
                            replica_groups=[list(range(NCORES))],
                            ins=[cc_in.opt()],
                            outs=[cc_out.opt()],
                        )
                else:
                    nc.sync.dma_start(cc_out[:], cc_in[0])

                # ---------------- FFN (token-parallel) ----------------
                with (
                    tc.tile_pool(name="ldp", bufs=2) as ldp,
                    tc.tile_pool(name="resp", bufs=1) as resp,
                    tc.tile_pool(name="hp", bufs=1) as hp,
                    tc.tile_pool(name="w2p", bufs=3) as w2p,
                    tc.tile_pool(name="outp", bufs=4) as outp,
                ):
                    res1 = []
                    res1b = []
                    for do in range(ND):
                        xg = ldp.tile([P, TOK], F32, tag="xg")
                        nc.sync.dma_start(xg[:], xTg_r[do])
                        co = ldp.tile([P, TOK], BF16, tag="co")
                        nc.scalar.dma_start(co[:], cc_out_r[do])
                        r1 = resp.tile([P, TOK], F32, tag=f"r1_{do}")
                        nc.vector.tensor_add(r1[:], xg[:], co[:])
                        res1.append(r1)
                        r1b = resp.tile([P, TOK], BF16, tag=f"r1b_{do}",
                                        name=f"r1b_{do}")
                        nc.vector.tensor_copy(r1b[:], r1[:])
                        res1b.append(r1b)

                    h_t = []
                    with tc.tile_pool(name="ps_f1", bufs=4,
                                      space="PSUM") as ps_f1:
                        for fo in range(NF):
                            hps = ps_f1.tile([P, TOK], F32, tag="f1")
                            fsl = slice(fo * P, (fo + 1) * P)
                            for do in range(ND):
                                nc.tensor.matmul(hps[:], w1_t[do][:, fsl],
                                                 res1b[do][:],
                                                 start=(do == 0),
                                                 stop=(do == ND - 1))
                            ht = hp.tile([P, TOK], BF16, tag=f"h_{fo}")
                            nc.scalar.activation(ht[:], hps[:], AF.Relu,
                                                 bias=b1_sb[:, fo:fo + 1])
                            h_t.append(ht)

                    with tc.tile_pool(name="ps_f2", bufs=1,
                                      space="PSUM") as ps_f2:
                        ops = [ps_f2.tile([P, TOK], F32, tag=f"f2_{do}",
                                          name=f"f2_{do}")
                               for do in range(ND)]
                        for fo in range(NF):
                            w2t = w2p.tile([P, D], BF16, tag="w2")
                            (nc.sync, nc.scalar)[fo % 2].dma_start(
                                w2t[:], w2b_in[fo])
                            for do in range(ND):
                                nc.tensor.matmul(
                                    ops[do][:], w2t[:, do * P:(do + 1) * P],
                                    h_t[fo][:],
                                    start=(fo == 0), stop=(fo == NF - 1))
                        for do in range(ND):
                            o2 = outp.tile([P, TOK], F32, tag="o2")
                            nc.vector.scalar_tensor_tensor(
                                o2[:], ops[do][:], b2_sb[:, do:do + 1],
                                res1[do][:], OP.add, OP.add)
                            (nc.sync, nc.scalar)[do % 2].dma_start(
                                outT_r[do], o2[:])

            for _rep in range(reps):
                emit_body()

    nc.compile()
    return nc


_CACHE = {}


def prepare_in_maps(encodings, Wq, Wk, Wv, W1, b1, W2, b2, mask):
    x = np.ascontiguousarray(np.asarray(encodings, dtype=np.float32))
    sched, mtiles = _mask_schedule(mask)
    n_mask = len(mtiles)

    xTb = np.ascontiguousarray(x.transpose(0, 2, 1)).astype(BF)   # [B, D, S]
    # x8[b, dp, p, j, s] = x[b, s, (2dp+j)*128+p]
    x8 = np.ascontiguousarray(
        x.reshape(B, S, ND2, 2, P).transpose(0, 2, 4, 3, 1)).astype(F8)
    w1b = np.ascontiguousarray(
        np.asarray(W1, np.float32).T.reshape(ND, P, DFF)).astype(BF)
    w2b = np.ascontiguousarray(
        np.asarray(W2, np.float32).T.reshape(NF, P, D)).astype(BF)
    b1c = np.ascontiguousarray(np.asarray(b1, np.float32).reshape(NF, P).T)
    b2c = np.ascontiguousarray(np.asarray(b2, np.float32).reshape(ND, P).T)
    mt = (np.stack(mtiles) if n_mask else np.zeros((1, P, SC), np.float32))
    mtb = mt.astype(BF)
    mt8 = mt.astype(F8)
    onecb = np.ones((P, P), BF)
    one8 = np.ones((P, 2, P), F8)

    xflat = x.reshape(B * S, D)
    in_maps = []
    for c in range(NCORES):
        wvT = np.ascontiguousarray(np.asarray(Wv[c], np.float32).T)  # [d, o]
        wv8 = np.ascontiguousarray(
            wvT.reshape(ND2, 2, P, D).transpose(0, 2, 1, 3)).astype(F8)
        in_maps.append({
            "xTb": xTb,
            "x8": x8,
            "wqT": np.ascontiguousarray(
                np.asarray(Wq[c], np.float32).T).astype(BF),
            "wkT": np.ascontiguousarray(
                np.asarray(Wk[c], np.float32).T).astype(BF),
            "wv8": wv8,
            "wvTb": wvT.astype(BF),
            "w1b": w1b,
            "w2b": w2b,
            "b1c": b1c,
            "b2c": b2c,
            "mtb": mtb,
            "mt8": mt8,
            "onecb": onecb,
            "one8": one8,
            "xTg": np.ascontiguousarray(xflat[c * TOK:(c + 1) * TOK].T),
        })
    return in_maps


def kernel(encodings, Wq, Wk, Wv, W1, b1, W2, b2, mask):
    from concourse.bass_utils import run_bass_kernel_spmd

    sched, mtiles = _mask_schedule(mask)
    n_mask = len(mtiles)
    key = (tuple(tuple(e) for e in sched), n_mask)
    if key not in _CACHE:
        _CACHE[key] = _build(sched, n_mask)
    nc = _CACHE[key]

    in_maps = prepare_in_maps(encodings, Wq, Wk, Wv, W1, b1, W2, b2, mask)

    res = run_bass_kernel_spmd(nc, in_maps, core_ids=list(range(NCORES)))
    out = np.empty((B * S, D), np.float32)
    for c in range(NCORES):
        out[c * TOK:(c + 1) * TOK] = res.results[c]["outT"].T
    kernel.last_results = res
    return out.reshape(B, S, D)

